# revision 14
# baseline (speedup 1.0000x reference)
"""AGCA (adaptive graph channel attention) distributed Bass kernel for TRN2.

Reference computation (per batch row b):
    y   = mean(x[b], axis=(H,W))                    # [CIN]
    y1  = W1 @ y                                    # [HIDE]
    A1  = softmax(w2 * y1)                          # [HIDE]
    y2  = y1 * A1 + A2.T-contract(y1)               # y1@A2
    y3  = relu(w3 * y2)
    out = sigmoid(W4 @ y3)                          # [OP]

Sharding: pure data-parallel over batch. Each of the 8 cores handles
B/8 = 8 batch rows end-to-end; the tiny params are replicated. No
collectives. The kernel is memory-bound on streaming x (64 MiB/core).

Per-core dataflow (v2: engine-15 tail-offload skew):
  - The SWDGE stream fans out across the 16 SDMA engines with the fixed
    map engine = partition mod 16. On this machine one core's engine 15
    runs ~0.79x the others (the known SWDGE descriptor-ring port
    contention on engines 7/15), which used to stretch that core's
    stream from 157us to ~196us.
  - Fix: every channel keeps only its first LA=3320 hw elements on its
    home partition (T1 [BL, CT, 128, 3320]); the 776-element tails of
    all 512 channels are packed onto partitions != 15 (mod 16) via
    per-range transfers (T2a: SBUF lanes 16k+0..14 for k=0..3, five
    776-slots per lane; T2b: same lanes for ranges 4..7, four slots).
    Engine 15 then moves 8*4*3320 elems/row vs 8*(4*3320+9*776)*...
    for the others -- a 0.79 ratio that matches its degraded rate, so
    all 16 engines finish together at ~161us.
  - Tail partial sums enter the same y1/y1T PSUM accumulation through
    9 extra matmul tiles whose W-rows are the duplicated W1 columns of
    the offloaded channels (zero rows for the 3 dead lanes per range
    group and the 28 pad slots). Dead SBUF lanes are pre-zeroed via
    tiny HWDGE DMAs from a zeros param so no NaN garbage can reach the
    matmul (garbage * 0 = NaN would poison PSUM).
  - The DMA casts f32 -> bf16 in the datapath (halves SBUF-AXI write
    traffic; per-engine read rate ~27 GB/s is the binding limit).
  - DVE/ACT sum-reduce the supertiles along the free axis into yt
    tiles (f32 accumulation; the 1/4096 mean scale is folded into the
    W tiles on the host). T1 body reduces alternate ACT/DVE; all T2
    reduces ride DVE (ACT's accum_out can only produce one column).
  - The final batch row's T1-ct3 data is tapered geometrically along
    hw so the post-last-byte reduce work is ~1us; T2 rows stream
    earlier, so the tail chain is unchanged.
  - Per channel tile, W1 matmuls run mid-stream on the tensor engine
    (y1 [8,128] and y1T [128,8] layouts both computed so softmax runs
    along the free axis). The epilogue reads y1 straight from PSUM:
    exp with fused accum (softmax denominator) on ACT, the
    normalize/A2/relu chain on DVE+PE, and sigmoid as
    0.5*tanh(z/2)+0.5 (tanh shares the exp LUT set).
  - Output [8, 512] (batch-major) DMAd out; host concatenates shards.
"""

import numpy as np

import concourse.bass as bass
import concourse.mybir as mybir
from concourse.bass_utils import run_bass_kernel_spmd


def _install_ntff_shim():
    """Fill in the optional antenv.axon_hooks module if the image lacks it,
    so run_bass_kernel_spmd(trace=True) (or BASS_TRACE=1) can drive NTFF
    profiling through libaxon_pjrt.so instead of crashing on the import.
    No-op when the module exists or the axon .so is unavailable."""
    import sys as _sys
    import types as _types

    if "antenv.axon_hooks" in _sys.modules:
        return
    try:
        import antenv  # noqa: F401
        import importlib.util as _ilu

        if _ilu.find_spec("antenv.axon_hooks") is not None:
            return
        mod = _types.ModuleType("antenv.axon_hooks")
        _hook = [None]
        mod.set_axon_ntff_profile_hook = lambda h: _hook.__setitem__(0, h)
        mod.get_axon_ntff_profile_hook = lambda: _hook[0]
        try:
            from trn_agent_boot.trn_boot import _ntff_profile_via_ctypes

            mod.set_axon_ntff_profile_hook(
                _ntff_profile_via_ctypes("/opt/axon/libaxon_pjrt.so")
            )
        except Exception:
            pass  # hook stays None; bass_utils logs and skips tracing
        _sys.modules["antenv.axon_hooks"] = mod
        antenv.axon_hooks = mod
    except Exception:
        pass


_install_ntff_shim()

F32 = mybir.dt.float32

B, CIN, H, W = 64, 512, 64, 64
HW = H * W          # 4096
NCORES = 8
BL = B // NCORES    # 8 batch rows per core
CT = CIN // 128     # 4 channel tiles
HIDE = 128
OP = 512
NBST = 2            # batch rows per (full) supertile
NBUF = 8            # T1 streaming buffers (bf16, 13.3 KiB/partition)

LA = 3320           # head length kept on the home partition
TT = HW - LA        # 776: offloaded tail length per channel
NT2A = 5            # tail slots per lane, ranges 0..3 (channels 0..299)
NT2B = 4            # tail slots per lane, ranges 4..7 (channels 300..511)
NLANE = 63          # T2 SBUF lanes 0..62 (lanes 15/31/47 dead = zero)

NTAPER = 5  # taper chunks for the very last batch row (1 ACT + 4 DVE)


def _taper_chunks(hw):
    ch = [hw // 2, hw // 4, hw // 8]
    rest = hw - sum(ch)
    ch += [rest // 2, rest - rest // 2]
    assert len(ch) == NTAPER and sum(ch) == hw
    return ch


def make_jobs(nbuf):
    """Streaming schedule.

    Job kinds:
      'x'  -- T1 supertile [128, nb, nhw] slice of the head tensor;
              dst ('yt', b0, nb) or ('ytx', k) for the taper.
      '2a'/'2b' -- T2 tail job: 4 range dma_starts + NT2 per-slot DVE
              reduces; dst ('yt2a'|'yt2b', b0, nb).
    Fields: b0, nb, ct (T1 only), hw0, nhw, eng ('V'/'A'), dst, slot,
    boff, sem (dma-sem index), wait (threshold), gate (job index whose
    consumer must finish before this DMA issues).
    """
    raw = []

    def add(**kw):
        raw.append(dict(kw))

    bi = 0  # T1 body cadence: alternate ACT/DVE (T2 jobs are DVE-only,
    # so T1 leans harder on ACT than the old every-3rd split)

    def beng():
        nonlocal bi
        e = 'A' if bi % 2 == 0 else 'V'
        bi += 1
        return e

    for ct in range(CT):
        if ct < CT - 1:
            for b0 in range(0, BL, NBST):
                add(kind='x', b0=b0, nb=NBST, ct=ct, hw0=0, nhw=LA,
                    eng=beng(), dst=('yt', b0, NBST))
            if ct == 1:
                for b0 in range(0, BL, NBST):
                    add(kind='2a', b0=b0, nb=NBST, eng='V',
                        dst=('yt2a', b0, NBST))
            if ct == 2:
                for b0 in range(0, BL, NBST):
                    add(kind='2b', b0=b0, nb=NBST, eng='V',
                        dst=('yt2b', b0, NBST))
        else:
            for b0 in range(0, BL - NBST, NBST):
                add(kind='x', b0=b0, nb=NBST, ct=ct, hw0=0, nhw=LA,
                    eng=beng(), dst=('yt', b0, NBST))
            add(kind='x', b0=BL - 2, nb=1, ct=ct, hw0=0, nhw=LA, eng='A',
                dst=('yt', BL - 2, 1))
            ch = _taper_chunks(LA)
            hw0 = 0
            for k in range(NTAPER):
                add(kind='x', b0=BL - 1, nb=1, ct=ct, hw0=hw0, nhw=ch[k],
                    eng='A' if k == 0 else 'V', dst=('ytx', k))
                hw0 += ch[k]

    # --- buffer slots, dma sems, gates ---
    # T1 body jobs rotate the nbuf slots; tail jobs (single + taper) use
    # slot 0 / slot 1 sub-regions with private sems. T2a/T2b rotate 2
    # buffers each with per-buffer sems (exact thresholds: a buffer's
    # next DMA only issues after the previous consumer ran).
    ntail = NTAPER + 1
    pos = {id(j): i for i, j in enumerate(raw)}
    t1_jobs = [j for j in raw if j['kind'] == 'x']
    t1_body = t1_jobs[:len(t1_jobs) - ntail]
    t1_tail = t1_jobs[len(t1_jobs) - ntail:]
    for i, j in enumerate(t1_body):
        j['slot'] = i % nbuf
        j['boff'] = 0
        j['sem'] = j['slot']
        j['wait'] = 16 * (i // nbuf + 1)
        j['gate'] = pos[id(t1_body[i - nbuf])] if i >= nbuf else None
    for t, j in enumerate(t1_tail):
        j['slot'] = 0 if t == 0 else 1
        j['boff'] = 0 if t == 0 else j['hw0']
        j['sem'] = nbuf + t
        j['wait'] = 16
        last_body = max(
            (i for i, jb in enumerate(t1_body) if jb['slot'] == j['slot'])
        )
        j['gate'] = pos[id(t1_body[last_body])]
    for kind, sem0 in (('2a', nbuf + ntail), ('2b', nbuf + ntail + 2)):
        js = [j for j in raw if j['kind'] == kind]
        for i, j in enumerate(js):
            j['slot'] = i % 2
            j['boff'] = 0
            j['sem'] = sem0 + j['slot']
            # 4 range transfers x16 per use of this buffer
            j['wait'] = 64 * (i // 2 + 1)
            j['gate'] = pos[id(js[i - 2])] if i >= 2 else None

    # --- producer (sem kind, cumulative count) per job ---
    # V jobs: T1 -> 1 reduce; T2a -> NT2A; T2b -> NT2B.
    # A jobs: one activation per batch row (nb increments).
    vcount = acount = 0
    for j in raw:
        if j['eng'] == 'V':
            vcount += {'x': 1, '2a': NT2A, '2b': NT2B}[j['kind']]
            j['prod'] = ('V', vcount)
        else:
            acount += j['nb']
            j['prod'] = ('A', acount)
    # completion counts for the matmul waits
    done = {}
    for key in ['ct0', 'ct1', 'ct2', '2a', '2b']:
        done[key] = [0, 0]
    for j in raw:
        key = f"ct{j['ct']}" if j['kind'] == 'x' and j['ct'] < CT - 1 else (
            j['kind'] if j['kind'] in ('2a', '2b') else None)
        if key is None:
            continue
        k, c = j['prod']
        idx = 0 if k == 'V' else 1
        done[key][idx] = max(done[key][idx], c)
    ndmasem = nbuf + ntail + 4
    return raw, done, vcount, acount, ntail, ndmasem


def build_nc(nbuf: int = NBUF):
    nc = bass.Bass(enable_partition_id=False, monotonic_sem_count=0)
    BF16 = mybir.dt.bfloat16

    x_e = nc.declare_dram_parameter("x", [BL, CT, 128, LA], F32, isOutput=False)
    t2a_e = nc.declare_dram_parameter(
        "t2a", [BL, 60, NT2A * TT], F32, isOutput=False)
    t2b_e = nc.declare_dram_parameter(
        "t2b", [BL, 60, NT2B * TT], F32, isOutput=False)
    w1t_e = nc.declare_dram_parameter("w1t", [128, CT, HIDE], F32, isOutput=False)
    w2a_e = nc.declare_dram_parameter(
        "w2a", [NLANE, NT2A, HIDE], F32, isOutput=False)
    w2b_e = nc.declare_dram_parameter(
        "w2b", [NLANE, NT2B, HIDE], F32, isOutput=False)
    zpad_e = nc.declare_dram_parameter(
        "zpad", [1, NBST * NT2A * TT], BF16, isOutput=False)
    a2_e = nc.declare_dram_parameter("a2", [HIDE, HIDE], BF16, isOutput=False)
    w4t_e = nc.declare_dram_parameter("w4t", [HIDE, OP], BF16, isOutput=False)
    scal_e = nc.declare_dram_parameter("scal", [BL, 2], F32, isOutput=False)
    eye_e = nc.declare_dram_parameter("eye8", [BL, BL], BF16, isOutput=False)
    out_e = nc.declare_dram_parameter("out", [BL, OP], F32, isOutput=True)

    Exp = mybir.ActivationFunctionType.Exp
    Tanh = mybir.ActivationFunctionType.Tanh
    Copy = mybir.ActivationFunctionType.Copy

    from contextlib import ExitStack

    with ExitStack() as ctx:
        bufs = [
            ctx.enter_context(nc.sbuf_tensor(f"buf{j}", [128, NBST, LA], BF16))
            for j in range(nbuf)
        ]
        b2a = [
            ctx.enter_context(
                nc.sbuf_tensor(f"b2a{j}", [NLANE, NBST, NT2A * TT], BF16))
            for j in range(2)
        ]
        b2b = [
            ctx.enter_context(
                nc.sbuf_tensor(f"b2b{j}", [NLANE, NBST, NT2B * TT], BF16))
            for j in range(2)
        ]
        yt = ctx.enter_context(nc.sbuf_tensor("yt", [128, CT, BL], F32))
        yt2a = ctx.enter_context(nc.sbuf_tensor("yt2a", [NLANE, NT2A, BL], F32))
        yt2b = ctx.enter_context(nc.sbuf_tensor("yt2b", [NLANE, NT2B, BL], F32))
        ytx = ctx.enter_context(nc.sbuf_tensor("ytx", [128, NTAPER], F32))
        waste = ctx.enter_context(nc.sbuf_tensor("waste", [128, 2, LA], BF16))
        w1ts = ctx.enter_context(nc.sbuf_tensor("w1ts", [128, CT, HIDE], F32))
        w2as = ctx.enter_context(
            nc.sbuf_tensor("w2as", [NLANE, NT2A, HIDE], F32))
        w2bs = ctx.enter_context(
            nc.sbuf_tensor("w2bs", [NLANE, NT2B, HIDE], F32))
        a2s = ctx.enter_context(nc.sbuf_tensor("a2s", [HIDE, HIDE], BF16))
        w4ts = ctx.enter_context(nc.sbuf_tensor("w4ts", [HIDE, OP], BF16))
        scals = ctx.enter_context(nc.sbuf_tensor("scals", [BL, 2], F32))
        eyes = ctx.enter_context(nc.sbuf_tensor("eyes", [BL, BL], BF16))
        de1 = ctx.enter_context(nc.sbuf_tensor("de1", [1, 1], F32))

        y1ts = ctx.enter_context(nc.sbuf_tensor("y1ts", [HIDE, BL], BF16))
        es = ctx.enter_context(nc.sbuf_tensor("es", [BL, HIDE], F32))
        ss = ctx.enter_context(nc.sbuf_tensor("ss", [BL, 1], F32))
        rs = ctx.enter_context(nc.sbuf_tensor("rs", [BL, 1], F32))
        t1s = ctx.enter_context(nc.sbuf_tensor("t1s", [BL, HIDE], F32))
        y2s = ctx.enter_context(nc.sbuf_tensor("y2s", [BL, HIDE], BF16))
        y3s = ctx.enter_context(nc.sbuf_tensor("y3s", [BL, HIDE], F32))
        y3ts = ctx.enter_context(nc.sbuf_tensor("y3ts", [HIDE, BL], BF16))
        esig = ctx.enter_context(nc.sbuf_tensor("esig", [BL, OP], F32))
        outs = ctx.enter_context(nc.sbuf_tensor("outs", [BL, OP], F32))

        y1_ps = ctx.enter_context(nc.psum_tensor("y1_ps", [BL, HIDE], F32))
        y1t_ps = ctx.enter_context(nc.psum_tensor("y1t_ps", [HIDE, BL], F32))
        p2_ps = ctx.enter_context(nc.psum_tensor("p2_ps", [BL, HIDE], F32))
        y3t_ps = ctx.enter_context(nc.psum_tensor("y3t_ps", [HIDE, BL], F32))
        o_ps = ctx.enter_context(nc.psum_tensor("o_ps", [BL, OP], F32))

        jobs, done, NV, NA, ntail, ndmasem = make_jobs(nbuf)
        R0 = NV + 1        # red_sem once yt complete (all V reduces + combine)
        AEXP = NA + 1      # act_sem count of the epilogue exp
        NPAIR = CT + NT2A + NT2B   # 13 matmul pairs into y1/y1T

        dma_sems = [
            ctx.enter_context(nc.semaphore(f"dma_sem{j}"))
            for j in range(ndmasem)
        ]
        out_sem = ctx.enter_context(nc.semaphore("out_sem"))
        param_sem = ctx.enter_context(nc.semaphore("param_sem"))
        red_sem = ctx.enter_context(nc.semaphore("red_sem"))
        pe_sem = ctx.enter_context(nc.semaphore("pe_sem"))
        act_sem = ctx.enter_context(nc.semaphore("act_sem"))
        sem_of = {'V': red_sem, 'A': act_sem}
        # 12 zero-pad DMAs (sync queue) inc out_sem by 16 each
        ZP = 12 * 16

        def buf_in(j):
            if j['kind'] == 'x':
                return bufs[j['slot']][:, 0:j['nb'],
                                       j['boff']:j['boff'] + j['nhw']]
            bl = b2a if j['kind'] == '2a' else b2b
            return bl[j['slot']][:, 0:j['nb'], :]

        def issue_stream(eng):
            for ji, j in enumerate(jobs):
                if j['gate'] is not None:
                    pk, pc = jobs[j['gate']]['prod']
                    eng.wait_ge(sem_of[pk], pc)
                if j['kind'] == 'x':
                    src = x_e[
                        j['b0']:j['b0'] + j['nb'], j['ct'], :,
                        j['hw0']:j['hw0'] + j['nhw']
                    ].rearrange("b p w -> p b w")
                    eng.dma_start(out=buf_in(j), in_=src).then_inc(
                        dma_sems[j['sem']], 16
                    )
                else:
                    te = t2a_e if j['kind'] == '2a' else t2b_e
                    bl = b2a if j['kind'] == '2a' else b2b
                    for k in range(4):
                        src = te[
                            j['b0']:j['b0'] + j['nb'], 15 * k:15 * k + 15, :
                        ].rearrange("b p w -> p b w")
                        eng.dma_start(
                            out=bl[j['slot']][16 * k:16 * k + 15,
                                              0:j['nb'], :],
                            in_=src,
                        ).then_inc(dma_sems[j['sem']], 16)

        with nc.Block() as block:

            @block.gpsimd
            def _(gpsimd):
                # SWDGE stream: casts f32 DRAM -> bf16 SBUF in the DMA
                # datapath, halving the SBUF-AXI write bytes.
                issue_stream(gpsimd)

            @block.sync
            def _(sync):
                # Zero the dead lanes (15/31/47) of the T2 buffers once;
                # they are never DMA-written, and garbage there would
                # reach the matmul as NaN*0.
                for bl in (b2a, b2b):
                    w = (NT2A if bl is b2a else NT2B) * TT
                    for t in bl:
                        for lane in (15, 31, 47):
                            sync.dma_start(
                                out=t[lane:lane + 1, :, :].rearrange(
                                    "p b w -> p (b w)"),
                                in_=zpad_e[:, 0:NBST * w],
                            ).then_inc(out_sem, 16)
                # Output DMA once both sigmoid halves land in SBUF.
                sync.wait_ge(red_sem, R0 + 5)
                sync.wait_ge(act_sem, AEXP + 4)
                sync.dma_start(out=out_e[:, :], in_=outs[:, :]).then_inc(
                    out_sem, 16)
                sync.wait_ge(out_sem, ZP + 16)

            @block.scalar
            def _(scalar):
                # Param loads lead the scalar HWDGE queue.
                scalar.dma_start(out=w1ts[:, :, :], in_=w1t_e[:, :, :]).then_inc(
                    param_sem, 16
                )
                scalar.dma_start(out=w2as[:, :, :], in_=w2a_e[:, :, :]).then_inc(
                    param_sem, 16
                )
                scalar.dma_start(out=w2bs[:, :, :], in_=w2b_e[:, :, :]).then_inc(
                    param_sem, 16
                )
                scalar.dma_start(out=a2s[:, :], in_=a2_e[:, :]).then_inc(
                    param_sem, 16)
                scalar.dma_start(out=w4ts[:, :], in_=w4t_e[:, :]).then_inc(
                    param_sem, 16
                )
                scalar.dma_start(out=scals[:, :], in_=scal_e[:, :]).then_inc(
                    param_sem, 16
                )
                scalar.dma_start(out=eyes[:, :], in_=eye_e[:, :]).then_inc(
                    param_sem, 16
                )
                # Preload the exp/tanh table set during the stream.
                c0 = nc.const_aps.tensor(0.0, (1, 1))
                scalar.activation(de1[:, :], c0, Exp)
                # Reduce assists: free-dim sums via accum_out, one call per
                # batch row. Two waste regions rotate; a self-wait orders the
                # region reuse for the pipeline.
                acalls = 0
                region_last = [0, 0]
                for j in jobs:
                    if j['eng'] != 'A':
                        continue
                    scalar.wait_ge(dma_sems[j['sem']], j['wait'])
                    for b in range(j['nb']):
                        reg = acalls % 2
                        if region_last[reg] > 0:
                            scalar.wait_ge(act_sem, region_last[reg])
                        acc = (
                            yt[:, j['ct'],
                               j['dst'][1] + b:j['dst'][1] + b + 1]
                            if j['dst'][0] == 'yt'
                            else ytx[:, j['dst'][1]:j['dst'][1] + 1]
                        )
                        scalar.activation(
                            waste[:, reg, 0:j['nhw']],
                            buf_in(j)[:, b, :],
                            Copy,
                            accum_out=acc,
                        ).then_inc(act_sem, 1)
                        acalls += 1
                        region_last[reg] = acalls
                # Epilogue: exp(w2*y1) with fused softmax denominator,
                # reading y1 straight out of PSUM.
                scalar.wait_ge(param_sem, 112)
                scalar.wait_ge(pe_sem, 2 * NPAIR - 1)
                scalar.activation(
                    es[:, :], y1_ps[:, :], Exp, scale=scals[:, 0:1],
                    accum_out=ss[:, :],
                ).then_inc(act_sem, 1)
                scalar.wait_ge(pe_sem, 2 * NPAIR + 2)
                scalar.activation(
                    y3ts[:, :], y3t_ps[:, :],
                    mybir.ActivationFunctionType.Relu,
                ).then_inc(act_sem, 1)
                # sigmoid(z) = 0.5*tanh(z/2) + 0.5 (tanh shares the exp
                # set). Column-half pipeline: ACT tanh h1, then tanh h2
                # while DVE applies h1's scale/bias; ACT finishes h2.
                scalar.wait_ge(pe_sem, 2 * NPAIR + 3)
                scalar.activation(
                    esig[:, 0:OP // 2], o_ps[:, 0:OP // 2], Tanh, scale=0.5
                ).then_inc(act_sem, 1)
                scalar.wait_ge(pe_sem, 2 * NPAIR + 4)
                scalar.activation(
                    esig[:, OP // 2:OP], o_ps[:, OP // 2:OP], Tanh, scale=0.5
                ).then_inc(act_sem, 1)
                scalar.wait_ge(act_sem, AEXP + 3)
                scalar.activation(
                    outs[:, OP // 2:OP], esig[:, OP // 2:OP], Copy,
                    scale=0.5, bias=0.5,
                ).then_inc(act_sem, 1)

            @block.vector
            def _(vector):
                first_t2 = True
                for j in jobs:
                    if j['eng'] != 'V':
                        continue
                    vector.wait_ge(dma_sems[j['sem']], j['wait'])
                    if j['kind'] == 'x':
                        out_ap = (
                            yt[:, j['ct'],
                               j['dst'][1]:j['dst'][1] + j['dst'][2]]
                            if j['dst'][0] == 'yt'
                            else ytx[:, j['dst'][1]:j['dst'][1] + 1]
                        )
                        vector.reduce_sum(
                            out_ap, buf_in(j), axis=mybir.AxisListType.X
                        ).then_inc(red_sem, 1)
                    else:
                        if first_t2:
                            # dead-lane zero pads must have landed
                            vector.wait_ge(out_sem, ZP)
                            first_t2 = False
                        ns = NT2A if j['kind'] == '2a' else NT2B
                        ytt = yt2a if j['kind'] == '2a' else yt2b
                        bl = b2a if j['kind'] == '2a' else b2b
                        for s in range(ns):
                            vector.reduce_sum(
                                ytt[:, s, j['b0']:j['b0'] + j['nb']],
                                bl[j['slot']][:, 0:j['nb'],
                                              s * TT:(s + 1) * TT],
                                axis=mybir.AxisListType.X,
                            ).then_inc(red_sem, 1)
                # Combine the taper partials: yt[:, CT-1, BL-1] = sum(ytx)
                vector.wait_ge(red_sem, NV)
                vector.wait_ge(act_sem, NA)
                vector.reduce_sum(
                    yt[:, CT - 1, BL - 1:BL], ytx[:, :],
                    axis=mybir.AxisListType.X,
                ).then_inc(red_sem, 1)
                # Epilogue. y1ts copy (f32->bf16) runs on DVE.
                vector.wait_ge(pe_sem, 2 * NPAIR)
                vector.tensor_copy(y1ts[:, :], y1t_ps[:, :]).then_inc(red_sem, 1)
                vector.wait_ge(act_sem, AEXP)
                vector.reciprocal(rs[:, :], ss[:, :]).then_inc(red_sem, 1)
                vector.wait_ge(red_sem, R0 + 2)
                # t1 = (es * 1/s) * y1  (y1 read from PSUM)
                vector.scalar_tensor_tensor(
                    t1s[:, :], es[:, :], rs[:, 0:1], y1_ps[:, :],
                    op0=mybir.AluOpType.mult, op1=mybir.AluOpType.mult,
                ).then_inc(red_sem, 1)
                vector.wait_ge(pe_sem, 2 * NPAIR + 1)
                vector.wait_ge(red_sem, R0 + 3)
                vector.tensor_add(y2s[:, :], t1s[:, :], p2_ps[:, :]).then_inc(
                    red_sem, 1
                )
                # Sigmoid tail, first half: outs_h1 = 0.5*tanh_h1 + 0.5
                vector.wait_ge(act_sem, AEXP + 2)
                vector.tensor_scalar(
                    outs[:, 0:OP // 2], esig[:, 0:OP // 2], 0.5, 0.5,
                    op0=mybir.AluOpType.mult, op1=mybir.AluOpType.add,
                ).then_inc(red_sem, 1)

            @block.tensor
            def _(tensor):
                tensor.wait_ge(param_sem, 112)
                # 13 matmul pairs accumulate y1 / y1T: T1 cts 0..2 as
                # their yt tiles complete, the 9 T2 tail tiles once all
                # T2 reduces are in, then T1 ct3 last (gated on the full
                # stream including the taper combine).
                pair = 0

                def mmpair(mov, stat, vwait=None, await_=None):
                    nonlocal pair
                    if vwait:
                        tensor.wait_ge(red_sem, vwait)
                    if await_:
                        tensor.wait_ge(act_sem, await_)
                    tensor.matmul(
                        y1_ps[:, :], mov, stat,
                        start=(pair == 0), stop=(pair == NPAIR - 1),
                    ).then_inc(pe_sem, 1)
                    tensor.matmul(
                        y1t_ps[:, :], stat, mov,
                        start=(pair == 0), stop=(pair == NPAIR - 1),
                    ).then_inc(pe_sem, 1)
                    pair += 1

                for ct in range(CT - 1):
                    v, a = done[f'ct{ct}']
                    mmpair(yt[:, ct, :], w1ts[:, ct, :],
                           vwait=v or None, await_=a or None)
                v, a = done['2a']
                tensor.wait_ge(red_sem, v)
                for s in range(NT2A):
                    mmpair(yt2a[:, s, :], w2as[:, s, :])
                v, a = done['2b']
                tensor.wait_ge(red_sem, v)
                for s in range(NT2B):
                    mmpair(yt2b[:, s, :], w2bs[:, s, :])
                mmpair(yt[:, CT - 1, :], w1ts[:, CT - 1, :], vwait=R0)
                # p2[b, k] = sum_h y1T[h, b] * A2[h, k]
                tensor.wait_ge(red_sem, R0 + 1)
                tensor.matmul(
                    p2_ps[:, :], y1ts[:, :], a2s[:, :], start=True, stop=True
                ).then_inc(pe_sem, 1)
                # w3*y2T via matmul with the w3-scaled identity
                tensor.wait_ge(red_sem, R0 + 4)
                tensor.matmul(
                    y3t_ps[:, :], y2s[:, :], eyes[:, :], start=True, stop=True
                ).then_inc(pe_sem, 1)
                # out[b, o] = sum_h y3T[h, b] * W4T[h, o], in column halves
                # so the sigmoid tail pipelines across ACT and DVE.
                tensor.wait_ge(act_sem, AEXP + 1)
                tensor.matmul(
                    o_ps[:, 0:OP // 2], y3ts[:, :], w4ts[:, 0:OP // 2],
                    start=True, stop=True, skip_group_check=True,
                ).then_inc(pe_sem, 1)
                tensor.matmul(
                    o_ps[:, OP // 2:OP], y3ts[:, :], w4ts[:, OP // 2:OP],
                    start=True, stop=True, skip_group_check=True,
                ).then_inc(pe_sem, 1)

    return nc


def prep_in_maps(x, W1, A2, w2, w3, W4):
    """Shard x over batch with the tail-offload skew; replicate params."""
    x = np.ascontiguousarray(np.asarray(x, dtype=np.float32))
    W1 = np.asarray(W1, np.float32)
    # W1T with the mean scale folded in: w1t[p, ct, h] = W1[h, ct*128+p]/hw
    w1t = np.ascontiguousarray(
        (W1.T / HW).reshape(CT, 128, HIDE).transpose(1, 0, 2)
    )
    # tail W tiles: lane l = 16k+r (r<15) <-> column q = 15k+r of the
    # 60-wide tail tensors; channel c = q*NT2A + s (T2a, c<300) or
    # 300 + q*NT2B + s (T2b, c<512; else zero pad row).
    w2a = np.zeros((NLANE, NT2A, HIDE), np.float32)
    w2b = np.zeros((NLANE, NT2B, HIDE), np.float32)
    for l in range(NLANE):
        k, r = divmod(l, 16)
        if r == 15:
            continue
        q = 15 * k + r
        for s in range(NT2A):
            c = q * NT2A + s
            if c < 300:
                w2a[l, s, :] = W1[:, c] / HW
        for s in range(NT2B):
            c = 300 + q * NT2B + s
            if c < 512:
                w2b[l, s, :] = W1[:, c] / HW
    import ml_dtypes

    a2 = np.ascontiguousarray(np.asarray(A2, np.float32)).astype(ml_dtypes.bfloat16)
    w4t = np.ascontiguousarray(np.asarray(W4, np.float32).T).astype(
        ml_dtypes.bfloat16
    )
    zpad = np.zeros((1, NBST * NT2A * TT), ml_dtypes.bfloat16)
    scal = np.empty((BL, 2), np.float32)
    scal[:, 0] = np.float32(w2)
    scal[:, 1] = np.float32(w3)
    # w3 folded into the transpose identity: the PE transpose-matmul then
    # produces w3*y2^T and the ACT copy applies relu.
    eye8 = (np.eye(BL) * np.float32(w3)).astype(ml_dtypes.bfloat16)

    in_maps = []
    for c in range(NCORES):
        xr = x[c * BL:(c + 1) * BL].reshape(BL, CT, 128, HW)
        t1 = np.ascontiguousarray(xr[..., :LA])
        # tails in channel-major order: c = ct*128 + p
        tails = np.ascontiguousarray(xr[..., LA:]).reshape(BL, CIN, TT)
        t2a = np.ascontiguousarray(tails[:, :300].reshape(BL, 60, NT2A * TT))
        t2b_pad = np.zeros((BL, 240, TT), np.float32)
        t2b_pad[:, :212] = tails[:, 300:512]
        t2b = np.ascontiguousarray(t2b_pad.reshape(BL, 60, NT2B * TT))
        in_maps.append(
            {
                "x": t1,
                "t2a": t2a,
                "t2b": t2b,
                "w1t": w1t,
                "w2a": w2a,
                "w2b": w2b,
                "zpad": zpad,
                "a2": a2,
                "w4t": w4t,
                "scal": scal,
                "eye8": eye8,
            }
        )
    return in_maps


def run(inputs: dict, trace: bool = False, tmpdir: str | None = None,
        trace_cores=None):
    """Build + run on 8 cores. Returns (full_output, BassKernelResults)."""
    nc = build_nc()
    in_maps = prep_in_maps(
        inputs["x"], inputs["W1"], inputs["A2"], inputs["w2"], inputs["w3"],
        inputs["W4"],
    )
    res = run_bass_kernel_spmd(
        nc, in_maps, core_ids=list(range(NCORES)), trace=trace, tmpdir=tmpdir,
        trace_cores=trace_cores,
    )
    out = np.concatenate([res.results[c]["out"] for c in range(NCORES)], axis=0)
    return out.reshape(B, OP, 1, 1).astype(np.float32), res


def kernel(**inputs) -> np.ndarray:
    out, _ = run(inputs, trace=False)
    return out


# revision 16
# speedup vs baseline: 1.1100x; 1.1100x over previous
"""AGCA (adaptive graph channel attention) distributed Bass kernel for TRN2.

Reference computation (per batch row b):
    y   = mean(x[b], axis=(H,W))                    # [CIN]
    y1  = W1 @ y                                    # [HIDE]
    A1  = softmax(w2 * y1)                          # [HIDE]
    y2  = y1 * A1 + A2.T-contract(y1)               # y1@A2
    y3  = relu(w3 * y2)
    out = sigmoid(W4 @ y3)                          # [OP]

Sharding: pure data-parallel over batch. Each of the 8 cores handles
B/8 = 8 batch rows end-to-end; the tiny params are replicated. No
collectives. The kernel is memory-bound on streaming x (64 MiB/core).

Per-core dataflow (v2: engine-15 tail-offload skew):
  - The SWDGE stream fans out across the 16 SDMA engines with the fixed
    map engine = partition mod 16. On this machine one core's engine 15
    runs ~0.79x the others (the known SWDGE descriptor-ring port
    contention on engines 7/15), which used to stretch that core's
    stream from 157us to ~196us.
  - Fix: every channel keeps only its first LA=3320 hw elements on its
    home partition (T1 [BL, CT, 128, 3320]); the 776-element tails of
    all 512 channels are packed onto partitions != 15 (mod 16) via
    per-range transfers (T2a: SBUF lanes 16k+0..14 for k=0..3, five
    776-slots per lane; T2b: same lanes for ranges 4..7, four slots).
    Engine 15 then moves 8*4*3320 elems/row vs 8*(4*3320+9*776)*...
    for the others -- a 0.79 ratio that matches its degraded rate, so
    all 16 engines finish together at ~161us.
  - Tail partial sums enter the same y1/y1T PSUM accumulation through
    9 extra matmul tiles whose W-rows are the duplicated W1 columns of
    the offloaded channels (zero rows for the 3 dead lanes per range
    group and the 28 pad slots). Dead SBUF lanes are pre-zeroed via
    tiny HWDGE DMAs from a zeros param so no NaN garbage can reach the
    matmul (garbage * 0 = NaN would poison PSUM).
  - The DMA casts f32 -> bf16 in the datapath (halves SBUF-AXI write
    traffic; per-engine read rate ~27 GB/s is the binding limit).
  - DVE/ACT sum-reduce the supertiles along the free axis into yt
    tiles (f32 accumulation; the 1/4096 mean scale is folded into the
    W tiles on the host). T1 body reduces alternate ACT/DVE; all T2
    reduces ride DVE (ACT's accum_out can only produce one column).
  - The final batch row's T1-ct3 data is tapered geometrically along
    hw so the post-last-byte reduce work is ~1us; T2 rows stream
    earlier, so the tail chain is unchanged.
  - Per channel tile, W1 matmuls run mid-stream on the tensor engine
    (y1 [8,128] and y1T [128,8] layouts both computed so softmax runs
    along the free axis). The epilogue reads y1 straight from PSUM:
    exp with fused accum (softmax denominator) on ACT, the
    normalize/A2/relu chain on DVE+PE, and sigmoid as
    0.5*tanh(z/2)+0.5 (tanh shares the exp LUT set).
  - Output [8, 512] (batch-major) DMAd out; host concatenates shards.
"""

import numpy as np

import concourse.bass as bass
import concourse.mybir as mybir
from concourse.bass_utils import run_bass_kernel_spmd


def _install_ntff_shim():
    """Fill in the optional antenv.axon_hooks module if the image lacks it,
    so run_bass_kernel_spmd(trace=True) (or BASS_TRACE=1) can drive NTFF
    profiling through libaxon_pjrt.so instead of crashing on the import.
    No-op when the module exists or the axon .so is unavailable."""
    import sys as _sys
    import types as _types

    if "antenv.axon_hooks" in _sys.modules:
        return
    try:
        import antenv  # noqa: F401
        import importlib.util as _ilu

        if _ilu.find_spec("antenv.axon_hooks") is not None:
            return
        mod = _types.ModuleType("antenv.axon_hooks")
        _hook = [None]
        mod.set_axon_ntff_profile_hook = lambda h: _hook.__setitem__(0, h)
        mod.get_axon_ntff_profile_hook = lambda: _hook[0]
        try:
            from trn_agent_boot.trn_boot import _ntff_profile_via_ctypes

            mod.set_axon_ntff_profile_hook(
                _ntff_profile_via_ctypes("/opt/axon/libaxon_pjrt.so")
            )
        except Exception:
            pass  # hook stays None; bass_utils logs and skips tracing
        _sys.modules["antenv.axon_hooks"] = mod
        antenv.axon_hooks = mod
    except Exception:
        pass


_install_ntff_shim()

F32 = mybir.dt.float32

B, CIN, H, W = 64, 512, 64, 64
HW = H * W          # 4096
NCORES = 8
BL = B // NCORES    # 8 batch rows per core
CT = CIN // 128     # 4 channel tiles
HIDE = 128
OP = 512
NBST = 2            # batch rows per (full) supertile
NBUF = 8            # T1 streaming buffers (bf16, 13.3 KiB/partition)

LA = 3328           # head length kept on the home partition (13312-byte
                    # rows, 512B-aligned: misaligned descriptor reads
                    # cost ~20% HBM efficiency)
TT = HW - LA        # 768: offloaded tail length per channel (3072B)
NT2A = 5            # tail slots per lane, ranges 0..3 (channels 0..299)
NT2B = 4            # tail slots per lane, ranges 4..7 (channels 300..511)
NLANE = 63          # T2 SBUF lanes 0..62 (lanes 15/31/47 dead = zero)

NTAPER = 5  # taper chunks for the very last batch row (1 ACT + 4 DVE)


def _taper_chunks(hw):
    ch = [hw // 2, hw // 4, hw // 8]
    rest = hw - sum(ch)
    ch += [rest // 2, rest - rest // 2]
    assert len(ch) == NTAPER and sum(ch) == hw
    return ch


def make_jobs(nbuf):
    """Streaming schedule.

    Job kinds:
      'x'  -- T1 supertile [128, nb, nhw] slice of the head tensor;
              dst ('yt', b0, nb) or ('ytx', k) for the taper.
      '2a'/'2b' -- T2 tail job: 4 range dma_starts + NT2 per-slot DVE
              reduces; dst ('yt2a'|'yt2b', b0, nb).
    Fields: b0, nb, ct (T1 only), hw0, nhw, eng ('V'/'A'), dst, slot,
    boff, sem (dma-sem index), wait (threshold), gate (job index whose
    consumer must finish before this DMA issues).
    """
    raw = []

    def add(**kw):
        raw.append(dict(kw))

    bi = 0  # T1 body cadence: alternate ACT/DVE (T2 jobs are DVE-only,
    # so T1 leans harder on ACT than the old every-3rd split)

    def beng():
        nonlocal bi
        e = 'A' if bi % 2 == 0 else 'V'
        bi += 1
        return e

    # T2 jobs are scattered one-or-two per ct block: their buffer-reuse
    # gates (job k waits job k-2's reduce) release DMA-paced, and the
    # in-order gpsimd issue loop must never hit a gate before the SDMA
    # rings hold enough buffered work to drain past the release time.
    t2_after = {0: [('2a', 0)], 1: [('2b', 0), ('2a', 2)],
                2: [('2b', 2), ('2a', 4)],
                3: [('2b', 4), ('2a', 6), ('2b', 6)]}
    for ct in range(CT):
        if ct < CT - 1:
            for b0 in range(0, BL, NBST):
                add(kind='x', b0=b0, nb=NBST, ct=ct, hw0=0, nhw=LA,
                    eng=beng(), dst=('yt', b0, NBST))
            for kind, b0 in t2_after[ct]:
                add(kind=kind, b0=b0, nb=NBST, eng='V',
                    dst=('yt' + kind, b0, NBST))
        else:
            for b0 in range(0, BL - NBST, NBST):
                add(kind='x', b0=b0, nb=NBST, ct=ct, hw0=0, nhw=LA,
                    eng=beng(), dst=('yt', b0, NBST))
            for kind, b0 in t2_after[ct]:
                add(kind=kind, b0=b0, nb=NBST, eng='V',
                    dst=('yt' + kind, b0, NBST))
            add(kind='x', b0=BL - 2, nb=1, ct=ct, hw0=0, nhw=LA, eng='A',
                dst=('yt', BL - 2, 1))
            ch = _taper_chunks(LA)
            hw0 = 0
            for k in range(NTAPER):
                add(kind='x', b0=BL - 1, nb=1, ct=ct, hw0=hw0, nhw=ch[k],
                    eng='A' if k == 0 else 'V', dst=('ytx', k))
                hw0 += ch[k]

    # --- buffer slots, dma sems, gates ---
    # T1 body jobs rotate the nbuf slots; tail jobs (single + taper) use
    # slot 0 / slot 1 sub-regions with private sems. T2a/T2b rotate 2
    # buffers each with per-buffer sems (exact thresholds: a buffer's
    # next DMA only issues after the previous consumer ran).
    ntail = NTAPER + 1
    pos = {id(j): i for i, j in enumerate(raw)}
    t1_jobs = [j for j in raw if j['kind'] == 'x']
    t1_body = t1_jobs[:len(t1_jobs) - ntail]
    t1_tail = t1_jobs[len(t1_jobs) - ntail:]
    for i, j in enumerate(t1_body):
        j['slot'] = i % nbuf
        j['boff'] = 0
        j['sem'] = j['slot']
        j['wait'] = 16 * (i // nbuf + 1)
        j['gate'] = pos[id(t1_body[i - nbuf])] if i >= nbuf else None
    for t, j in enumerate(t1_tail):
        j['slot'] = 0 if t == 0 else 1
        j['boff'] = 0 if t == 0 else j['hw0']
        j['sem'] = nbuf + t
        j['wait'] = 16
        last_body = max(
            (i for i, jb in enumerate(t1_body) if jb['slot'] == j['slot'])
        )
        j['gate'] = pos[id(t1_body[last_body])]
    for kind, sem0 in (('2a', nbuf + ntail), ('2b', nbuf + ntail + 2)):
        js = [j for j in raw if j['kind'] == kind]
        for i, j in enumerate(js):
            j['slot'] = i % 2
            j['boff'] = 0
            j['sem'] = sem0 + j['slot']
            # 4 range transfers x16 per use of this buffer
            j['wait'] = 64 * (i // 2 + 1)
            j['gate'] = pos[id(js[i - 2])] if i >= 2 else None

    # --- producer (sem kind, cumulative count) per job ---
    # V jobs: T1 -> 1 reduce; T2a -> NT2A; T2b -> NT2B.
    # A jobs: one activation per batch row (nb increments).
    vcount = acount = 0
    for j in raw:
        if j['eng'] == 'V':
            vcount += {'x': 1, '2a': NT2A, '2b': NT2B}[j['kind']]
            j['prod'] = ('V', vcount)
        else:
            acount += j['nb']
            j['prod'] = ('A', acount)
    # completion counts for the matmul waits
    done = {}
    for key in ['ct0', 'ct1', 'ct2', '2a', '2b']:
        done[key] = [0, 0]
    for j in raw:
        key = f"ct{j['ct']}" if j['kind'] == 'x' and j['ct'] < CT - 1 else (
            j['kind'] if j['kind'] in ('2a', '2b') else None)
        if key is None:
            continue
        k, c = j['prod']
        idx = 0 if k == 'V' else 1
        done[key][idx] = max(done[key][idx], c)
    ndmasem = nbuf + ntail + 4
    return raw, done, vcount, acount, ntail, ndmasem


def build_nc(nbuf: int = NBUF):
    nc = bass.Bass(enable_partition_id=False, monotonic_sem_count=0)
    BF16 = mybir.dt.bfloat16

    x_e = nc.declare_dram_parameter("x", [BL, CT, 128, LA], F32, isOutput=False)
    t2a_e = nc.declare_dram_parameter(
        "t2a", [BL, 60, NT2A * TT], F32, isOutput=False)
    t2b_e = nc.declare_dram_parameter(
        "t2b", [BL, 60, NT2B * TT], F32, isOutput=False)
    w1t_e = nc.declare_dram_parameter("w1t", [128, CT, HIDE], F32, isOutput=False)
    w2a_e = nc.declare_dram_parameter(
        "w2a", [NLANE, NT2A, HIDE], F32, isOutput=False)
    w2b_e = nc.declare_dram_parameter(
        "w2b", [NLANE, NT2B, HIDE], F32, isOutput=False)
    zpad_e = nc.declare_dram_parameter(
        "zpad", [1, NBST * NT2A * TT], BF16, isOutput=False)
    a2_e = nc.declare_dram_parameter("a2", [HIDE, HIDE], BF16, isOutput=False)
    w4t_e = nc.declare_dram_parameter("w4t", [HIDE, OP], BF16, isOutput=False)
    scal_e = nc.declare_dram_parameter("scal", [BL, 2], F32, isOutput=False)
    eye_e = nc.declare_dram_parameter("eye8", [BL, BL], BF16, isOutput=False)
    out_e = nc.declare_dram_parameter("out", [BL, OP], F32, isOutput=True)

    Exp = mybir.ActivationFunctionType.Exp
    Tanh = mybir.ActivationFunctionType.Tanh
    Copy = mybir.ActivationFunctionType.Copy

    from contextlib import ExitStack

    with ExitStack() as ctx:
        bufs = [
            ctx.enter_context(nc.sbuf_tensor(f"buf{j}", [128, NBST, LA], BF16))
            for j in range(nbuf)
        ]
        b2a = [
            ctx.enter_context(
                nc.sbuf_tensor(f"b2a{j}", [NLANE, NBST, NT2A * TT], BF16))
            for j in range(2)
        ]
        b2b = [
            ctx.enter_context(
                nc.sbuf_tensor(f"b2b{j}", [NLANE, NBST, NT2B * TT], BF16))
            for j in range(2)
        ]
        yt = ctx.enter_context(nc.sbuf_tensor("yt", [128, CT, BL], F32))
        yt2a = ctx.enter_context(nc.sbuf_tensor("yt2a", [NLANE, NT2A, BL], F32))
        yt2b = ctx.enter_context(nc.sbuf_tensor("yt2b", [NLANE, NT2B, BL], F32))
        ytx = ctx.enter_context(nc.sbuf_tensor("ytx", [128, NTAPER], F32))
        waste = ctx.enter_context(nc.sbuf_tensor("waste", [128, 2, LA], BF16))
        w1ts = ctx.enter_context(nc.sbuf_tensor("w1ts", [128, CT, HIDE], F32))
        w2as = ctx.enter_context(
            nc.sbuf_tensor("w2as", [NLANE, NT2A, HIDE], F32))
        w2bs = ctx.enter_context(
            nc.sbuf_tensor("w2bs", [NLANE, NT2B, HIDE], F32))
        a2s = ctx.enter_context(nc.sbuf_tensor("a2s", [HIDE, HIDE], BF16))
        w4ts = ctx.enter_context(nc.sbuf_tensor("w4ts", [HIDE, OP], BF16))
        scals = ctx.enter_context(nc.sbuf_tensor("scals", [BL, 2], F32))
        eyes = ctx.enter_context(nc.sbuf_tensor("eyes", [BL, BL], BF16))
        de1 = ctx.enter_context(nc.sbuf_tensor("de1", [1, 1], F32))

        y1ts = ctx.enter_context(nc.sbuf_tensor("y1ts", [HIDE, BL], BF16))
        es = ctx.enter_context(nc.sbuf_tensor("es", [BL, HIDE], F32))
        ss = ctx.enter_context(nc.sbuf_tensor("ss", [BL, 1], F32))
        rs = ctx.enter_context(nc.sbuf_tensor("rs", [BL, 1], F32))
        t1s = ctx.enter_context(nc.sbuf_tensor("t1s", [BL, HIDE], F32))
        y2s = ctx.enter_context(nc.sbuf_tensor("y2s", [BL, HIDE], BF16))
        y3s = ctx.enter_context(nc.sbuf_tensor("y3s", [BL, HIDE], F32))
        y3ts = ctx.enter_context(nc.sbuf_tensor("y3ts", [HIDE, BL], BF16))
        esig = ctx.enter_context(nc.sbuf_tensor("esig", [BL, OP], F32))
        outs = ctx.enter_context(nc.sbuf_tensor("outs", [BL, OP], F32))

        y1_ps = ctx.enter_context(nc.psum_tensor("y1_ps", [BL, HIDE], F32))
        y1t_ps = ctx.enter_context(nc.psum_tensor("y1t_ps", [HIDE, BL], F32))
        p2_ps = ctx.enter_context(nc.psum_tensor("p2_ps", [BL, HIDE], F32))
        y3t_ps = ctx.enter_context(nc.psum_tensor("y3t_ps", [HIDE, BL], F32))
        o_ps = ctx.enter_context(nc.psum_tensor("o_ps", [BL, OP], F32))

        jobs, done, NV, NA, ntail, ndmasem = make_jobs(nbuf)
        R0 = NV + 1        # red_sem once yt complete (all V reduces + combine)
        AEXP = NA + 1      # act_sem count of the epilogue exp
        NPAIR = CT + NT2A + NT2B   # 13 matmul pairs into y1/y1T

        dma_sems = [
            ctx.enter_context(nc.semaphore(f"dma_sem{j}"))
            for j in range(ndmasem)
        ]
        out_sem = ctx.enter_context(nc.semaphore("out_sem"))
        param_sem = ctx.enter_context(nc.semaphore("param_sem"))
        red_sem = ctx.enter_context(nc.semaphore("red_sem"))
        pe_sem = ctx.enter_context(nc.semaphore("pe_sem"))
        act_sem = ctx.enter_context(nc.semaphore("act_sem"))
        sem_of = {'V': red_sem, 'A': act_sem}
        # 12 zero-pad DMAs (sync queue) inc out_sem by 16 each
        ZP = 12 * 16

        def buf_in(j):
            if j['kind'] == 'x':
                return bufs[j['slot']][:, 0:j['nb'],
                                       j['boff']:j['boff'] + j['nhw']]
            bl = b2a if j['kind'] == '2a' else b2b
            return bl[j['slot']][:, 0:j['nb'], :]

        def issue_stream(eng):
            for ji, j in enumerate(jobs):
                if j['gate'] is not None:
                    pk, pc = jobs[j['gate']]['prod']
                    eng.wait_ge(sem_of[pk], pc)
                if j['kind'] == 'x':
                    src = x_e[
                        j['b0']:j['b0'] + j['nb'], j['ct'], :,
                        j['hw0']:j['hw0'] + j['nhw']
                    ].rearrange("b p w -> p b w")
                    eng.dma_start(out=buf_in(j), in_=src).then_inc(
                        dma_sems[j['sem']], 16
                    )
                else:
                    te = t2a_e if j['kind'] == '2a' else t2b_e
                    bl = b2a if j['kind'] == '2a' else b2b
                    for k in range(4):
                        src = te[
                            j['b0']:j['b0'] + j['nb'], 15 * k:15 * k + 15, :
                        ].rearrange("b p w -> p b w")
                        eng.dma_start(
                            out=bl[j['slot']][16 * k:16 * k + 15,
                                              0:j['nb'], :],
                            in_=src,
                        ).then_inc(dma_sems[j['sem']], 16)

        with nc.Block() as block:

            @block.gpsimd
            def _(gpsimd):
                # SWDGE stream: casts f32 DRAM -> bf16 SBUF in the DMA
                # datapath, halving the SBUF-AXI write bytes.
                issue_stream(gpsimd)

            @block.sync
            def _(sync):
                # Zero the dead lanes (15/31/47) of the T2 buffers once;
                # they are never DMA-written, and garbage there would
                # reach the matmul as NaN*0.
                for bl in (b2a, b2b):
                    w = (NT2A if bl is b2a else NT2B) * TT
                    for t in bl:
                        for lane in (15, 31, 47):
                            sync.dma_start(
                                out=t[lane:lane + 1, :, :].rearrange(
                                    "p b w -> p (b w)"),
                                in_=zpad_e[:, 0:NBST * w],
                            ).then_inc(out_sem, 16)
                # Output DMA once both sigmoid halves land in SBUF.
                sync.wait_ge(red_sem, R0 + 5)
                sync.wait_ge(act_sem, AEXP + 4)
                sync.dma_start(out=out_e[:, :], in_=outs[:, :]).then_inc(
                    out_sem, 16)
                sync.wait_ge(out_sem, ZP + 16)

            @block.scalar
            def _(scalar):
                # Param loads lead the scalar HWDGE queue.
                scalar.dma_start(out=w1ts[:, :, :], in_=w1t_e[:, :, :]).then_inc(
                    param_sem, 16
                )
                scalar.dma_start(out=w2as[:, :, :], in_=w2a_e[:, :, :]).then_inc(
                    param_sem, 16
                )
                scalar.dma_start(out=w2bs[:, :, :], in_=w2b_e[:, :, :]).then_inc(
                    param_sem, 16
                )
                scalar.dma_start(out=a2s[:, :], in_=a2_e[:, :]).then_inc(
                    param_sem, 16)
                scalar.dma_start(out=w4ts[:, :], in_=w4t_e[:, :]).then_inc(
                    param_sem, 16
                )
                scalar.dma_start(out=scals[:, :], in_=scal_e[:, :]).then_inc(
                    param_sem, 16
                )
                scalar.dma_start(out=eyes[:, :], in_=eye_e[:, :]).then_inc(
                    param_sem, 16
                )
                # Preload the exp/tanh table set during the stream.
                c0 = nc.const_aps.tensor(0.0, (1, 1))
                scalar.activation(de1[:, :], c0, Exp)
                # Reduce assists: free-dim sums via accum_out, one call per
                # batch row. Two waste regions rotate; a self-wait orders the
                # region reuse for the pipeline.
                acalls = 0
                region_last = [0, 0]
                for j in jobs:
                    if j['eng'] != 'A':
                        continue
                    scalar.wait_ge(dma_sems[j['sem']], j['wait'])
                    for b in range(j['nb']):
                        reg = acalls % 2
                        if region_last[reg] > 0:
                            scalar.wait_ge(act_sem, region_last[reg])
                        acc = (
                            yt[:, j['ct'],
                               j['dst'][1] + b:j['dst'][1] + b + 1]
                            if j['dst'][0] == 'yt'
                            else ytx[:, j['dst'][1]:j['dst'][1] + 1]
                        )
                        scalar.activation(
                            waste[:, reg, 0:j['nhw']],
                            buf_in(j)[:, b, :],
                            Copy,
                            accum_out=acc,
                        ).then_inc(act_sem, 1)
                        acalls += 1
                        region_last[reg] = acalls
                # Epilogue: exp(w2*y1) with fused softmax denominator,
                # reading y1 straight out of PSUM.
                scalar.wait_ge(param_sem, 112)
                scalar.wait_ge(pe_sem, 2 * NPAIR - 1)
                scalar.activation(
                    es[:, :], y1_ps[:, :], Exp, scale=scals[:, 0:1],
                    accum_out=ss[:, :],
                ).then_inc(act_sem, 1)
                scalar.wait_ge(pe_sem, 2 * NPAIR + 2)
                scalar.activation(
                    y3ts[:, :], y3t_ps[:, :],
                    mybir.ActivationFunctionType.Relu,
                ).then_inc(act_sem, 1)
                # sigmoid(z) = 0.5*tanh(z/2) + 0.5 (tanh shares the exp
                # set). Column-half pipeline: ACT tanh h1, then tanh h2
                # while DVE applies h1's scale/bias; ACT finishes h2.
                scalar.wait_ge(pe_sem, 2 * NPAIR + 3)
                scalar.activation(
                    esig[:, 0:OP // 2], o_ps[:, 0:OP // 2], Tanh, scale=0.5
                ).then_inc(act_sem, 1)
                scalar.wait_ge(pe_sem, 2 * NPAIR + 4)
                scalar.activation(
                    esig[:, OP // 2:OP], o_ps[:, OP // 2:OP], Tanh, scale=0.5
                ).then_inc(act_sem, 1)
                scalar.wait_ge(act_sem, AEXP + 3)
                scalar.activation(
                    outs[:, OP // 2:OP], esig[:, OP // 2:OP], Copy,
                    scale=0.5, bias=0.5,
                ).then_inc(act_sem, 1)

            @block.vector
            def _(vector):
                first_t2 = True
                for j in jobs:
                    if j['eng'] != 'V':
                        continue
                    vector.wait_ge(dma_sems[j['sem']], j['wait'])
                    if j['kind'] == 'x':
                        out_ap = (
                            yt[:, j['ct'],
                               j['dst'][1]:j['dst'][1] + j['dst'][2]]
                            if j['dst'][0] == 'yt'
                            else ytx[:, j['dst'][1]:j['dst'][1] + 1]
                        )
                        vector.reduce_sum(
                            out_ap, buf_in(j), axis=mybir.AxisListType.X
                        ).then_inc(red_sem, 1)
                    else:
                        if first_t2:
                            # dead-lane zero pads must have landed
                            vector.wait_ge(out_sem, ZP)
                            first_t2 = False
                        ns = NT2A if j['kind'] == '2a' else NT2B
                        ytt = yt2a if j['kind'] == '2a' else yt2b
                        bl = b2a if j['kind'] == '2a' else b2b
                        for s in range(ns):
                            vector.reduce_sum(
                                ytt[:, s, j['b0']:j['b0'] + j['nb']],
                                bl[j['slot']][:, 0:j['nb'],
                                              s * TT:(s + 1) * TT],
                                axis=mybir.AxisListType.X,
                            ).then_inc(red_sem, 1)
                # Combine the taper partials: yt[:, CT-1, BL-1] = sum(ytx)
                vector.wait_ge(red_sem, NV)
                vector.wait_ge(act_sem, NA)
                vector.reduce_sum(
                    yt[:, CT - 1, BL - 1:BL], ytx[:, :],
                    axis=mybir.AxisListType.X,
                ).then_inc(red_sem, 1)
                # Epilogue. y1ts copy (f32->bf16) runs on DVE.
                vector.wait_ge(pe_sem, 2 * NPAIR)
                vector.tensor_copy(y1ts[:, :], y1t_ps[:, :]).then_inc(red_sem, 1)
                vector.wait_ge(act_sem, AEXP)
                vector.reciprocal(rs[:, :], ss[:, :]).then_inc(red_sem, 1)
                vector.wait_ge(red_sem, R0 + 2)
                # t1 = (es * 1/s) * y1  (y1 read from PSUM)
                vector.scalar_tensor_tensor(
                    t1s[:, :], es[:, :], rs[:, 0:1], y1_ps[:, :],
                    op0=mybir.AluOpType.mult, op1=mybir.AluOpType.mult,
                ).then_inc(red_sem, 1)
                vector.wait_ge(pe_sem, 2 * NPAIR + 1)
                vector.wait_ge(red_sem, R0 + 3)
                vector.tensor_add(y2s[:, :], t1s[:, :], p2_ps[:, :]).then_inc(
                    red_sem, 1
                )
                # Sigmoid tail, first half: outs_h1 = 0.5*tanh_h1 + 0.5
                vector.wait_ge(act_sem, AEXP + 2)
                vector.tensor_scalar(
                    outs[:, 0:OP // 2], esig[:, 0:OP // 2], 0.5, 0.5,
                    op0=mybir.AluOpType.mult, op1=mybir.AluOpType.add,
                ).then_inc(red_sem, 1)

            @block.tensor
            def _(tensor):
                tensor.wait_ge(param_sem, 112)
                # 13 matmul pairs accumulate y1 / y1T: T1 cts 0..2 as
                # their yt tiles complete, the 9 T2 tail tiles once all
                # T2 reduces are in, then T1 ct3 last (gated on the full
                # stream including the taper combine).
                pair = 0

                def mmpair(mov, stat, vwait=None, await_=None):
                    nonlocal pair
                    if vwait:
                        tensor.wait_ge(red_sem, vwait)
                    if await_:
                        tensor.wait_ge(act_sem, await_)
                    tensor.matmul(
                        y1_ps[:, :], mov, stat,
                        start=(pair == 0), stop=(pair == NPAIR - 1),
                    ).then_inc(pe_sem, 1)
                    tensor.matmul(
                        y1t_ps[:, :], stat, mov,
                        start=(pair == 0), stop=(pair == NPAIR - 1),
                    ).then_inc(pe_sem, 1)
                    pair += 1

                for ct in range(CT - 1):
                    v, a = done[f'ct{ct}']
                    mmpair(yt[:, ct, :], w1ts[:, ct, :],
                           vwait=v or None, await_=a or None)
                v, a = done['2a']
                tensor.wait_ge(red_sem, v)
                for s in range(NT2A):
                    mmpair(yt2a[:, s, :], w2as[:, s, :])
                v, a = done['2b']
                tensor.wait_ge(red_sem, v)
                for s in range(NT2B):
                    mmpair(yt2b[:, s, :], w2bs[:, s, :])
                mmpair(yt[:, CT - 1, :], w1ts[:, CT - 1, :], vwait=R0)
                # p2[b, k] = sum_h y1T[h, b] * A2[h, k]
                tensor.wait_ge(red_sem, R0 + 1)
                tensor.matmul(
                    p2_ps[:, :], y1ts[:, :], a2s[:, :], start=True, stop=True
                ).then_inc(pe_sem, 1)
                # w3*y2T via matmul with the w3-scaled identity
                tensor.wait_ge(red_sem, R0 + 4)
                tensor.matmul(
                    y3t_ps[:, :], y2s[:, :], eyes[:, :], start=True, stop=True
                ).then_inc(pe_sem, 1)
                # out[b, o] = sum_h y3T[h, b] * W4T[h, o], in column halves
                # so the sigmoid tail pipelines across ACT and DVE.
                tensor.wait_ge(act_sem, AEXP + 1)
                tensor.matmul(
                    o_ps[:, 0:OP // 2], y3ts[:, :], w4ts[:, 0:OP // 2],
                    start=True, stop=True, skip_group_check=True,
                ).then_inc(pe_sem, 1)
                tensor.matmul(
                    o_ps[:, OP // 2:OP], y3ts[:, :], w4ts[:, OP // 2:OP],
                    start=True, stop=True, skip_group_check=True,
                ).then_inc(pe_sem, 1)

    return nc


def prep_in_maps(x, W1, A2, w2, w3, W4):
    """Shard x over batch with the tail-offload skew; replicate params."""
    x = np.ascontiguousarray(np.asarray(x, dtype=np.float32))
    W1 = np.asarray(W1, np.float32)
    # W1T with the mean scale folded in: w1t[p, ct, h] = W1[h, ct*128+p]/hw
    w1t = np.ascontiguousarray(
        (W1.T / HW).reshape(CT, 128, HIDE).transpose(1, 0, 2)
    )
    # tail W tiles: lane l = 16k+r (r<15) <-> column q = 15k+r of the
    # 60-wide tail tensors; channel c = q*NT2A + s (T2a, c<300) or
    # 300 + q*NT2B + s (T2b, c<512; else zero pad row).
    w2a = np.zeros((NLANE, NT2A, HIDE), np.float32)
    w2b = np.zeros((NLANE, NT2B, HIDE), np.float32)
    for l in range(NLANE):
        k, r = divmod(l, 16)
        if r == 15:
            continue
        q = 15 * k + r
        for s in range(NT2A):
            c = q * NT2A + s
            if c < 300:
                w2a[l, s, :] = W1[:, c] / HW
        for s in range(NT2B):
            c = 300 + q * NT2B + s
            if c < 512:
                w2b[l, s, :] = W1[:, c] / HW
    import ml_dtypes

    a2 = np.ascontiguousarray(np.asarray(A2, np.float32)).astype(ml_dtypes.bfloat16)
    w4t = np.ascontiguousarray(np.asarray(W4, np.float32).T).astype(
        ml_dtypes.bfloat16
    )
    zpad = np.zeros((1, NBST * NT2A * TT), ml_dtypes.bfloat16)
    scal = np.empty((BL, 2), np.float32)
    scal[:, 0] = np.float32(w2)
    scal[:, 1] = np.float32(w3)
    # w3 folded into the transpose identity: the PE transpose-matmul then
    # produces w3*y2^T and the ACT copy applies relu.
    eye8 = (np.eye(BL) * np.float32(w3)).astype(ml_dtypes.bfloat16)

    in_maps = []
    for c in range(NCORES):
        xr = x[c * BL:(c + 1) * BL].reshape(BL, CT, 128, HW)
        t1 = np.ascontiguousarray(xr[..., :LA])
        # tails in channel-major order: c = ct*128 + p
        tails = np.ascontiguousarray(xr[..., LA:]).reshape(BL, CIN, TT)
        t2a = np.ascontiguousarray(tails[:, :300].reshape(BL, 60, NT2A * TT))
        t2b_pad = np.zeros((BL, 240, TT), np.float32)
        t2b_pad[:, :212] = tails[:, 300:512]
        t2b = np.ascontiguousarray(t2b_pad.reshape(BL, 60, NT2B * TT))
        in_maps.append(
            {
                "x": t1,
                "t2a": t2a,
                "t2b": t2b,
                "w1t": w1t,
                "w2a": w2a,
                "w2b": w2b,
                "zpad": zpad,
                "a2": a2,
                "w4t": w4t,
                "scal": scal,
                "eye8": eye8,
            }
        )
    return in_maps


def run(inputs: dict, trace: bool = False, tmpdir: str | None = None,
        trace_cores=None):
    """Build + run on 8 cores. Returns (full_output, BassKernelResults)."""
    nc = build_nc()
    in_maps = prep_in_maps(
        inputs["x"], inputs["W1"], inputs["A2"], inputs["w2"], inputs["w3"],
        inputs["W4"],
    )
    res = run_bass_kernel_spmd(
        nc, in_maps, core_ids=list(range(NCORES)), trace=trace, tmpdir=tmpdir,
        trace_cores=trace_cores,
    )
    out = np.concatenate([res.results[c]["out"] for c in range(NCORES)], axis=0)
    return out.reshape(B, OP, 1, 1).astype(np.float32), res


def kernel(**inputs) -> np.ndarray:
    out, _ = run(inputs, trace=False)
    return out


# revision 22
# speedup vs baseline: 1.1554x; 1.0409x over previous
"""AGCA (adaptive graph channel attention) distributed Bass kernel for TRN2.

Reference computation (per batch row b):
    y   = mean(x[b], axis=(H,W))                    # [CIN]
    y1  = W1 @ y                                    # [HIDE]
    A1  = softmax(w2 * y1)                          # [HIDE]
    y2  = y1 * A1 + A2.T-contract(y1)               # y1@A2
    y3  = relu(w3 * y2)
    out = sigmoid(W4 @ y3)                          # [OP]

Sharding: pure data-parallel over batch. Each of the 8 cores handles
B/8 = 8 batch rows end-to-end; the tiny params are replicated. No
collectives. The kernel is memory-bound on streaming x (64 MiB/core).

Per-core dataflow:
  - x shard viewed as [BL=8, CT=4, 128, 4096] (batch, channel-tile,
    channel-within-tile, H*W), streamed ct-major as ~4 MiB supertiles
    through an NBUF-deep SBUF rotation via SWDGE DMA that casts
    f32 -> bf16 in the datapath (halves SBUF-AXI write traffic; the
    per-SDMA-engine read rate ~27 GB/s x 16 engines is the binding
    limit).
  - SDMA engine balancing: descriptors are dealt to the 16 SDMA
    engines by a persistent round-robin over the global descriptor
    sequence (engine = desc_index mod 16; partition placement is
    irrelevant). On this machine one core's engine 15 runs at ~0.79x
    the others (the known SWDGE descriptor-ring contention on engines
    7/15), which used to stretch that core's stream ~25%. Fix: 4 of
    the 16 supertiles are issued "paired": per 16-partition group, a
    15-desc transfer (full 4096-elem rows -> engines 0..14) plus a
    1-desc transfer (first 256 elems of the 16th row -> engine 15),
    then one 16-desc transfer carrying the eight [256:4096] row
    remainders (engine 15 gets exactly one). Every transfer's desc
    count is phase-aligned mod 16, so engine 15 receives only the
    short descriptors: its stream bytes drop 22% to match its speed,
    and all 16 engines finish together. The SBUF tile written is
    bit-identical to the uniform path, so reduces/matmuls/epilogue
    are untouched.
  - The vector engine sum-reduces each supertile along the free axis
    into yT tiles [128c, 8b] (f32 accumulation; the 1/4096 mean scale
    is folded into W1 on the host). Every 3rd body reduce and the two
    slowest tail reduces ride the otherwise-idle scalar engine
    (activation Copy with accum_out). The final batch row is tapered
    geometrically along hw, so the post-last-byte reduce work is ~1us.
  - Per channel tile, its W1 matmuls run mid-stream on the tensor
    engine (y1 [8,128] and y1T [128,8] layouts both computed so softmax
    runs along the free axis). The epilogue reads y1 straight from
    PSUM: exp with fused accum (softmax denominator) on ACT, the
    normalize/A2/relu chain on DVE+PE, and sigmoid as
    0.5*tanh(z/2)+0.5 (tanh shares the exp LUT set, so no table load
    sits on the critical path).
  - Output [8, 512] (batch-major) DMAd out; host concatenates shards.
"""

import numpy as np

import concourse.bass as bass
import concourse.mybir as mybir
from concourse.bass_utils import run_bass_kernel_spmd


def _install_ntff_shim():
    """Fill in the optional antenv.axon_hooks module if the image lacks it,
    so run_bass_kernel_spmd(trace=True) (or BASS_TRACE=1) can drive NTFF
    profiling through libaxon_pjrt.so instead of crashing on the import.
    No-op when the module exists or the axon .so is unavailable."""
    import sys as _sys
    import types as _types

    if "antenv.axon_hooks" in _sys.modules:
        return
    try:
        import antenv  # noqa: F401
        import importlib.util as _ilu

        if _ilu.find_spec("antenv.axon_hooks") is not None:
            return
        mod = _types.ModuleType("antenv.axon_hooks")
        _hook = [None]
        mod.set_axon_ntff_profile_hook = lambda h: _hook.__setitem__(0, h)
        mod.get_axon_ntff_profile_hook = lambda: _hook[0]
        try:
            from trn_agent_boot.trn_boot import _ntff_profile_via_ctypes

            mod.set_axon_ntff_profile_hook(
                _ntff_profile_via_ctypes("/opt/axon/libaxon_pjrt.so")
            )
        except Exception:
            pass  # hook stays None; bass_utils logs and skips tracing
        _sys.modules["antenv.axon_hooks"] = mod
        antenv.axon_hooks = mod
    except Exception:
        pass


_install_ntff_shim()

F32 = mybir.dt.float32

B, CIN, H, W = 64, 512, 64, 64
HW = H * W          # 4096
NCORES = 8
BL = B // NCORES    # 8 batch rows per core
CT = CIN // 128     # 4 channel tiles
HIDE = 128
OP = 512
NBST = 2            # batch rows per (full) supertile
NBUF = 8            # streaming buffers (bf16: ample SBUF; deep
                    # lookahead decouples DMA issue from reduce
                    # completion in throttled epochs)

LS = 256            # short-row length fed to engine 15 in paired units
NPAIRED = 4         # paired supertiles (one per ct)

NTAPER = 5  # taper chunks for the very last batch row (1 ACT + 4 DVE)


def make_jobs(hw, nbuf):
    """Streaming schedule.

    Each job dict: b0, nb, ct, hw0, nhw, eng ('V' DVE / 'A' ACT reduce),
    dst ('yt', b0, nb) | ('ytx', k), slot (buffer index), boff (element
    offset within the slot's hw axis), sem (completion-sem index), wait
    (threshold), gate (job index whose consumer must finish before this
    DMA issues), paired (engine-15-skew issue pattern).

    Body jobs rotate through the nbuf slots. Tail jobs (b=BL-1 taper +
    the b=BL-2 single) use slot 0 / slot 1 sub-regions with private
    sems so nothing gates on late reduces.
    """
    raw = []

    def add(**kw):
        kw.setdefault('paired', False)
        raw.append(dict(kw))

    bi = 0  # body index; every 3rd body reduce rides ACT so the DVE
    # keeps a wide margin even when engine clocks are throttled ~20%.

    def beng():
        nonlocal bi
        e = 'A' if bi % 3 == 0 else 'V'
        bi += 1
        return e

    # One paired supertile per ct, never the tail pair (b6/b7).
    paired_units = {(0, 0), (1, 2), (2, 4), (3, 0)}
    for ct in range(CT):
        if ct < CT - 1:
            for b0 in range(0, BL, NBST):
                add(b0=b0, nb=NBST, ct=ct, hw0=0, nhw=hw, eng=beng(),
                    dst=('yt', b0, NBST), paired=(ct, b0) in paired_units)
        else:
            for b0 in range(0, BL - NBST, NBST):
                add(b0=b0, nb=NBST, ct=ct, hw0=0, nhw=hw, eng=beng(),
                    dst=('yt', b0, NBST), paired=(ct, b0) in paired_units)
            add(b0=BL - 2, nb=1, ct=ct, hw0=0, nhw=hw, eng='A',
                dst=('yt', BL - 2, 1))
            ch = [hw // 2, hw // 4, hw // 8, hw // 16, hw // 16]
            assert len(ch) == NTAPER and sum(ch) == hw
            hw0 = 0
            for k in range(NTAPER):
                add(b0=BL - 1, nb=1, ct=ct, hw0=hw0, nhw=ch[k],
                    eng='A' if k == 0 else 'V', dst=('ytx', k))
                hw0 += ch[k]

    ntail = NTAPER + 1
    nbody = len(raw) - ntail
    # a paired job issues 16 A + 16 B + 8 C = 40 transfers, each
    # incrementing the slot sem by 16
    ntr = lambda j: 40 if j['paired'] else 1
    sem_cum = [0] * (nbuf + ntail)
    for i, j in enumerate(raw):
        if i < nbody:
            j['slot'] = i % nbuf
            j['boff'] = 0
            j['sem'] = j['slot']
            sem_cum[j['sem']] += 16 * ntr(j)
            j['wait'] = sem_cum[j['sem']]
            j['gate'] = i - nbuf if i >= nbuf else None
        else:
            t = i - nbody
            j['slot'] = 0 if t == 0 else 1
            j['boff'] = 0 if t == 0 else j['hw0']
            j['sem'] = nbuf + t
            j['wait'] = 16
            # gate on the consumer of that slot's last body occupant
            last_body = max(b for b in range(nbody) if b % nbuf == j['slot'])
            j['gate'] = last_body

    # producer (sem kind, cumulative count) per job + per-ct counts.
    # An ACT job issues one activation per batch row (accum_out is one
    # column), so it increments act_sem nb times.
    vcount = acount = 0
    ct_vdone = [0] * CT
    ct_adone = [0] * CT
    for j in raw:
        if j['eng'] == 'V':
            vcount += 1
            j['prod'] = ('V', vcount)
        else:
            acount += j['nb']
            j['prod'] = ('A', acount)
        ct_vdone[j['ct']] = vcount
        ct_adone[j['ct']] = acount
    return raw, ct_vdone, ct_adone, vcount, acount, ntail


def build_nc(hw: int = HW, nbuf: int = NBUF):
    nc = bass.Bass(enable_partition_id=False, monotonic_sem_count=0)
    BF16 = mybir.dt.bfloat16

    x_e = nc.declare_dram_parameter("x", [BL, CT, 128, hw], F32, isOutput=False)
    w1t_e = nc.declare_dram_parameter("w1t", [128, CT, HIDE], F32, isOutput=False)
    a2_e = nc.declare_dram_parameter("a2", [HIDE, HIDE], BF16, isOutput=False)
    w4t_e = nc.declare_dram_parameter("w4t", [HIDE, OP], BF16, isOutput=False)
    scal_e = nc.declare_dram_parameter("scal", [BL, 2], F32, isOutput=False)
    eye_e = nc.declare_dram_parameter("eye8", [BL, BL], BF16, isOutput=False)
    out_e = nc.declare_dram_parameter("out", [BL, OP], F32, isOutput=True)

    Exp = mybir.ActivationFunctionType.Exp
    Tanh = mybir.ActivationFunctionType.Tanh
    Copy = mybir.ActivationFunctionType.Copy

    from contextlib import ExitStack

    with ExitStack() as ctx:
        bufs = [
            ctx.enter_context(nc.sbuf_tensor(f"buf{j}", [128, NBST, hw], BF16))
            for j in range(nbuf)
        ]
        yt = ctx.enter_context(nc.sbuf_tensor("yt", [128, CT, BL], F32))
        ytx = ctx.enter_context(nc.sbuf_tensor("ytx", [128, NTAPER], F32))
        waste = ctx.enter_context(
            nc.sbuf_tensor("waste", [128, 2, hw], BF16)
        )
        w1ts = ctx.enter_context(nc.sbuf_tensor("w1ts", [128, CT, HIDE], F32))
        a2s = ctx.enter_context(nc.sbuf_tensor("a2s", [HIDE, HIDE], BF16))
        w4ts = ctx.enter_context(nc.sbuf_tensor("w4ts", [HIDE, OP], BF16))
        scals = ctx.enter_context(nc.sbuf_tensor("scals", [BL, 2], F32))
        eyes = ctx.enter_context(nc.sbuf_tensor("eyes", [BL, BL], BF16))
        de1 = ctx.enter_context(nc.sbuf_tensor("de1", [1, 1], F32))

        y1ts = ctx.enter_context(nc.sbuf_tensor("y1ts", [HIDE, BL], BF16))
        es = ctx.enter_context(nc.sbuf_tensor("es", [BL, HIDE], F32))
        ss = ctx.enter_context(nc.sbuf_tensor("ss", [BL, 1], F32))
        rs = ctx.enter_context(nc.sbuf_tensor("rs", [BL, 1], F32))
        t1s = ctx.enter_context(nc.sbuf_tensor("t1s", [BL, HIDE], F32))
        y2s = ctx.enter_context(nc.sbuf_tensor("y2s", [BL, HIDE], BF16))
        y3s = ctx.enter_context(nc.sbuf_tensor("y3s", [BL, HIDE], F32))
        y3ts = ctx.enter_context(nc.sbuf_tensor("y3ts", [HIDE, BL], BF16))
        esig = ctx.enter_context(nc.sbuf_tensor("esig", [BL, OP], F32))
        outs = ctx.enter_context(nc.sbuf_tensor("outs", [BL, OP], F32))

        y1_ps = ctx.enter_context(nc.psum_tensor("y1_ps", [BL, HIDE], F32))
        y1t_ps = ctx.enter_context(nc.psum_tensor("y1t_ps", [HIDE, BL], F32))
        p2_ps = ctx.enter_context(nc.psum_tensor("p2_ps", [BL, HIDE], F32))
        y3t_ps = ctx.enter_context(nc.psum_tensor("y3t_ps", [HIDE, BL], F32))
        o_ps = ctx.enter_context(nc.psum_tensor("o_ps", [BL, OP], F32))

        jobs, ct_vdone, ct_adone, NV, NA, ntail = make_jobs(hw, nbuf)
        R0 = NV + 1        # red_sem once yt complete (all V reduces + combine)
        AEXP = NA + 1      # act_sem count of the epilogue exp

        dma_sems = [
            ctx.enter_context(nc.semaphore(f"dma_sem{j}"))
            for j in range(nbuf + ntail)
        ]
        out_sem = ctx.enter_context(nc.semaphore("out_sem"))
        param_sem = ctx.enter_context(nc.semaphore("param_sem"))
        red_sem = ctx.enter_context(nc.semaphore("red_sem"))
        pe_sem = ctx.enter_context(nc.semaphore("pe_sem"))
        act_sem = ctx.enter_context(nc.semaphore("act_sem"))
        sem_of = {'V': red_sem, 'A': act_sem}

        def buf_in(j):
            return bufs[j['slot']][
                :, 0:j['nb'], j['boff']:j['boff'] + j['nhw']
            ]

        def issue_paired(eng, j, sem):
            """40 phase-aligned transfers filling the same [128, nb, hw]
            tile as the uniform path: per (row, group): [15, 1, hw] to
            engines 0-14 and [1, 1, LS] to engine 15, then 8 two-desc
            transfers with the [LS:hw] remainders of the 8 short rows."""
            bf = bufs[j['slot']]
            for bi in range(j['nb']):
                b = j['b0'] + bi
                for g in range(8):
                    eng.dma_start(
                        out=bf[16 * g:16 * g + 15, bi:bi + 1, :],
                        in_=x_e[b:b + 1, j['ct'], 16 * g:16 * g + 15, :]
                        .rearrange("b p w -> p b w"),
                    ).then_inc(sem, 16)
                    eng.dma_start(
                        out=bf[16 * g + 15:16 * g + 16, bi:bi + 1, 0:LS],
                        in_=x_e[b:b + 1, j['ct'], 16 * g + 15:16 * g + 16,
                                0:LS].rearrange("b p w -> p b w"),
                    ).then_inc(sem, 16)
            # remainders of the 8 short rows: 8 x [1, nb, hw-LS]
            # (2 descs each -> phases 2g, 2g+1; engine 15 gets exactly
            # the last one)
            for g in range(8):
                p = 16 * g + 15
                eng.dma_start(
                    out=bf[p:p + 1, 0:j['nb'], LS:],
                    in_=x_e[j['b0']:j['b0'] + j['nb'], j['ct'],
                            p:p + 1, LS:].rearrange("b p w -> p b w"),
                ).then_inc(sem, 16)

        def issue_stream(eng):
            for j in jobs:
                if j['gate'] is not None:
                    pk, pc = jobs[j['gate']]['prod']
                    eng.wait_ge(sem_of[pk], pc)
                if j['paired']:
                    issue_paired(eng, j, dma_sems[j['sem']])
                    continue
                src = x_e[
                    j['b0']:j['b0'] + j['nb'], j['ct'], :,
                    j['hw0']:j['hw0'] + j['nhw']
                ].rearrange("b p w -> p b w")
                eng.dma_start(out=buf_in(j), in_=src).then_inc(
                    dma_sems[j['sem']], 16
                )

        with nc.Block() as block:

            @block.gpsimd
            def _(gpsimd):
                # SWDGE stream: casts f32 DRAM -> bf16 SBUF in the DMA
                # datapath, halving the SBUF-AXI write bytes.
                issue_stream(gpsimd)

            @block.sync
            def _(sync):
                # Output DMA once both sigmoid halves land in SBUF.
                sync.wait_ge(red_sem, R0 + 5)
                sync.wait_ge(act_sem, AEXP + 4)
                sync.dma_start(out=out_e[:, :], in_=outs[:, :]).then_inc(out_sem, 16)
                sync.wait_ge(out_sem, 16)

            @block.scalar
            def _(scalar):
                # Param loads lead the scalar HWDGE queue.
                scalar.dma_start(out=w1ts[:, :, :], in_=w1t_e[:, :, :]).then_inc(
                    param_sem, 16
                )
                scalar.dma_start(out=a2s[:, :], in_=a2_e[:, :]).then_inc(param_sem, 16)
                scalar.dma_start(out=w4ts[:, :], in_=w4t_e[:, :]).then_inc(
                    param_sem, 16
                )
                scalar.dma_start(out=scals[:, :], in_=scal_e[:, :]).then_inc(
                    param_sem, 16
                )
                scalar.dma_start(out=eyes[:, :], in_=eye_e[:, :]).then_inc(
                    param_sem, 16
                )
                # Preload the exp/tanh table set during the stream.
                c0 = nc.const_aps.tensor(0.0, (1, 1))
                scalar.activation(de1[:, :], c0, Exp)
                # Reduce assists: free-dim sums via accum_out, one call per
                # batch row. Two waste regions rotate; a self-wait orders the
                # region reuse for the pipeline.
                acalls = 0
                region_last = [0, 0]
                for j in jobs:
                    if j['eng'] != 'A':
                        continue
                    scalar.wait_ge(dma_sems[j['sem']], j['wait'])
                    for b in range(j['nb']):
                        reg = acalls % 2
                        if region_last[reg] > 0:
                            scalar.wait_ge(act_sem, region_last[reg])
                        acc = (
                            yt[:, j['ct'],
                               j['dst'][1] + b:j['dst'][1] + b + 1]
                            if j['dst'][0] == 'yt'
                            else ytx[:, j['dst'][1]:j['dst'][1] + 1]
                        )
                        scalar.activation(
                            waste[:, reg, 0:j['nhw']],
                            buf_in(j)[:, b, :],
                            Copy,
                            accum_out=acc,
                        ).then_inc(act_sem, 1)
                        acalls += 1
                        region_last[reg] = acalls
                # Epilogue: exp(w2*y1) with fused softmax denominator,
                # reading y1 straight out of PSUM.
                scalar.wait_ge(param_sem, 80)
                scalar.wait_ge(pe_sem, 7)
                scalar.activation(
                    es[:, :], y1_ps[:, :], Exp, scale=scals[:, 0:1],
                    accum_out=ss[:, :],
                ).then_inc(act_sem, 1)
                scalar.wait_ge(pe_sem, 10)
                scalar.activation(
                    y3ts[:, :], y3t_ps[:, :],
                    mybir.ActivationFunctionType.Relu,
                ).then_inc(act_sem, 1)
                # sigmoid(z) = 0.5*tanh(z/2) + 0.5 (tanh shares the exp
                # set). Column-half pipeline: ACT tanh h1, then tanh h2
                # while DVE applies h1's scale/bias; ACT finishes h2.
                scalar.wait_ge(pe_sem, 11)
                scalar.activation(
                    esig[:, 0:OP // 2], o_ps[:, 0:OP // 2], Tanh, scale=0.5
                ).then_inc(act_sem, 1)
                scalar.wait_ge(pe_sem, 12)
                scalar.activation(
                    esig[:, OP // 2:OP], o_ps[:, OP // 2:OP], Tanh, scale=0.5
                ).then_inc(act_sem, 1)
                scalar.wait_ge(act_sem, AEXP + 3)
                scalar.activation(
                    outs[:, OP // 2:OP], esig[:, OP // 2:OP], Copy,
                    scale=0.5, bias=0.5,
                ).then_inc(act_sem, 1)

            @block.vector
            def _(vector):
                for j in jobs:
                    if j['eng'] != 'V':
                        continue
                    vector.wait_ge(dma_sems[j['sem']], j['wait'])
                    out_ap = (
                        yt[:, j['ct'], j['dst'][1]:j['dst'][1] + j['dst'][2]]
                        if j['dst'][0] == 'yt'
                        else ytx[:, j['dst'][1]:j['dst'][1] + 1]
                    )
                    vector.reduce_sum(
                        out_ap, buf_in(j), axis=mybir.AxisListType.X
                    ).then_inc(red_sem, 1)
                # Combine the taper partials: yt[:, CT-1, BL-1] = sum(ytx)
                vector.wait_ge(red_sem, NV)
                vector.wait_ge(act_sem, NA)
                vector.reduce_sum(
                    yt[:, CT - 1, BL - 1:BL], ytx[:, :],
                    axis=mybir.AxisListType.X,
                ).then_inc(red_sem, 1)
                # Epilogue. y1ts copy (f32->bf16) runs on DVE.
                vector.wait_ge(pe_sem, 8)
                vector.tensor_copy(y1ts[:, :], y1t_ps[:, :]).then_inc(red_sem, 1)
                vector.wait_ge(act_sem, AEXP)
                vector.reciprocal(rs[:, :], ss[:, :]).then_inc(red_sem, 1)
                vector.wait_ge(red_sem, R0 + 2)
                # t1 = (es * 1/s) * y1  (y1 read from PSUM)
                vector.scalar_tensor_tensor(
                    t1s[:, :], es[:, :], rs[:, 0:1], y1_ps[:, :],
                    op0=mybir.AluOpType.mult, op1=mybir.AluOpType.mult,
                ).then_inc(red_sem, 1)
                vector.wait_ge(pe_sem, 9)
                vector.wait_ge(red_sem, R0 + 3)
                vector.tensor_add(y2s[:, :], t1s[:, :], p2_ps[:, :]).then_inc(
                    red_sem, 1
                )
                # Sigmoid tail, first half: outs_h1 = 0.5*tanh_h1 + 0.5
                vector.wait_ge(act_sem, AEXP + 2)
                vector.tensor_scalar(
                    outs[:, 0:OP // 2], esig[:, 0:OP // 2], 0.5, 0.5,
                    op0=mybir.AluOpType.mult, op1=mybir.AluOpType.add,
                ).then_inc(red_sem, 1)

            @block.tensor
            def _(tensor):
                tensor.wait_ge(param_sem, 80)
                # W1 matmuls per channel tile, issued as soon as that tile of
                # yt is fully reduced (overlaps the remaining stream).
                for ct in range(CT):
                    if ct < CT - 1:
                        tensor.wait_ge(red_sem, ct_vdone[ct])
                        if ct_adone[ct] > 0:
                            tensor.wait_ge(act_sem, ct_adone[ct])
                    else:
                        tensor.wait_ge(red_sem, R0)
                    tensor.matmul(
                        y1_ps[:, :],
                        yt[:, ct, :],
                        w1ts[:, ct, :],
                        start=(ct == 0),
                        stop=(ct == CT - 1),
                    ).then_inc(pe_sem, 1)
                    tensor.matmul(
                        y1t_ps[:, :],
                        w1ts[:, ct, :],
                        yt[:, ct, :],
                        start=(ct == 0),
                        stop=(ct == CT - 1),
                    ).then_inc(pe_sem, 1)
                # p2[b, k] = sum_h y1T[h, b] * A2[h, k]
                tensor.wait_ge(red_sem, R0 + 1)
                tensor.matmul(
                    p2_ps[:, :], y1ts[:, :], a2s[:, :], start=True, stop=True
                ).then_inc(pe_sem, 1)
                # w3*y2T via matmul with the w3-scaled identity
                tensor.wait_ge(red_sem, R0 + 4)
                tensor.matmul(
                    y3t_ps[:, :], y2s[:, :], eyes[:, :], start=True, stop=True
                ).then_inc(pe_sem, 1)
                # out[b, o] = sum_h y3T[h, b] * W4T[h, o], in column halves
                # so the sigmoid tail pipelines across ACT and DVE.
                tensor.wait_ge(act_sem, AEXP + 1)
                tensor.matmul(
                    o_ps[:, 0:OP // 2], y3ts[:, :], w4ts[:, 0:OP // 2],
                    start=True, stop=True, skip_group_check=True,
                ).then_inc(pe_sem, 1)
                tensor.matmul(
                    o_ps[:, OP // 2:OP], y3ts[:, :], w4ts[:, OP // 2:OP],
                    start=True, stop=True, skip_group_check=True,
                ).then_inc(pe_sem, 1)

    return nc


def prep_in_maps(x, W1, A2, w2, w3, W4, hw: int = HW):
    """Shard x over batch; replicate (pre-transposed) params."""
    x = np.ascontiguousarray(np.asarray(x, dtype=np.float32))
    # W1T with the mean scale folded in: [c, h] -> [128, CT, HIDE] with
    # w1t[p, ct, h] = W1[h, ct*128+p] / hw
    w1t = np.ascontiguousarray(
        (np.asarray(W1, np.float32).T / hw).reshape(CT, 128, HIDE).transpose(1, 0, 2)
    )
    import ml_dtypes

    a2 = np.ascontiguousarray(np.asarray(A2, np.float32)).astype(ml_dtypes.bfloat16)
    w4t = np.ascontiguousarray(np.asarray(W4, np.float32).T).astype(
        ml_dtypes.bfloat16
    )
    scal = np.empty((BL, 2), np.float32)
    scal[:, 0] = np.float32(w2)
    scal[:, 1] = np.float32(w3)
    # w3 folded into the transpose identity: the PE transpose-matmul then
    # produces w3*y2^T and the ACT copy applies relu.
    eye8 = (np.eye(BL) * np.float32(w3)).astype(ml_dtypes.bfloat16)

    in_maps = []
    for c in range(NCORES):
        xs = x[c * BL:(c + 1) * BL].reshape(BL, CT, 128, hw)
        in_maps.append(
            {
                "x": xs,
                "w1t": w1t,
                "a2": a2,
                "w4t": w4t,
                "scal": scal,
                "eye8": eye8,
            }
        )
    return in_maps


def run(inputs: dict, trace: bool = False, tmpdir: str | None = None,
        trace_cores=None):
    """Build + run on 8 cores. Returns (full_output, BassKernelResults)."""
    nc = build_nc()
    in_maps = prep_in_maps(
        inputs["x"], inputs["W1"], inputs["A2"], inputs["w2"], inputs["w3"],
        inputs["W4"],
    )
    res = run_bass_kernel_spmd(
        nc, in_maps, core_ids=list(range(NCORES)), trace=trace, tmpdir=tmpdir,
        trace_cores=trace_cores,
    )
    out = np.concatenate([res.results[c]["out"] for c in range(NCORES)], axis=0)
    return out.reshape(B, OP, 1, 1).astype(np.float32), res


def kernel(**inputs) -> np.ndarray:
    out, _ = run(inputs, trace=False)
    return out
